# revision 12
# baseline (speedup 1.0000x reference)
"""Trainium2 Bass kernel for AdaptiveLRLinearWithChannel (moe_routing).

Reference math:
    w    = (weights_U[indices] @ weights_V).reshape(B, IN, OUT)
    out  = einsum('bni,bio->bno', x, w) + bias[indices]

Strategy (8 NeuronCores, data-parallel over B):
  - Shard B=256 into 8 x 32 batches; U/V/bias are tiny and are folded on host
    into per-batch weight matrices W[b] and bias rows (host marshalling only;
    all O(B*N*IN*OUT) FLOPs run on device).
  - Host re-lays x out as xT[b] = x[b].T so the contraction dim (IN) lands on
    SBUF partitions, and casts x/W to bf16 (matmul accumulates in f32 PSUM;
    rel err ~3e-3).
  - Per core, per batch: one merged DMA loads xT[b] ([128, 2*2048]: both
    K-chunks side by side), one loads W[b]; for each pair of n-tiles a full
    PSUM bank [128, 512] accumulates 4 matmuls (2 K-chunks x 2 n-tiles); DVE
    adds the (partition-replicated) bias while evacuating PSUM -> SBUF as
    bf16; stores go out on the Activation HWDGE ring (loads use the SP ring),
    4 n-tiles per DMA. Output upcast to f32 on host.
"""

import sys

for _p in ("/opt/trn_rl_repo",):
    if _p not in sys.path:
        sys.path.insert(0, _p)

import numpy as np

B = 256
N = 2048
IN_SZ = 256
OUT_SZ = 256
N_CORES = 8
BPC = B // N_CORES  # 32 batches per core
NT = N // 128  # 16 n-tiles per batch
GROUP = 8  # n-tiles per output DMA
PSW = 2  # n-tiles per PSUM bank

_CACHE = {}


def _bf16():
    import ml_dtypes

    return ml_dtypes.bfloat16


def _emit_body(nc, xT, w, out, bias_sb, xp, wp, op, psum):
    import concourse.mybir as mybir

    bf16 = mybir.dt.bfloat16
    f32 = mybir.dt.float32

    for b in range(BPC):
        # xt[p, c*N + n] = xT[b, 128c + p, n]   (c = K-chunk)
        xt = xp.tile([128, 2 * N], bf16, tag="xt")
        nc.sync.dma_start(
            out=xt[:].rearrange("p (c n) -> p c n", c=2),
            in_=xT[b].rearrange("(c p) n -> p c n", p=128),
        )
        # wt[p, c*OUT + o] = w[b, 128c + p, o]
        wt = wp.tile([128, 2 * OUT_SZ], bf16, tag="wt")
        nc.sync.dma_start(
            out=wt[:].rearrange("p (c o) -> p c o", c=2),
            in_=w[b].rearrange("(c p) o -> p c o", p=128),
        )
        bias3 = bias_sb[:, b * OUT_SZ : (b + 1) * OUT_SZ][:, None, :].broadcast_to(
            [128, PSW, OUT_SZ]
        )

        for g in range(NT // GROUP):
            og = op.tile([128, GROUP * OUT_SZ], bf16, tag="og")
            for u in range(GROUP // PSW):
                ps = psum.tile([128, PSW * OUT_SZ], f32, tag="ps")
                for v in range(PSW):
                    t = g * GROUP + u * PSW + v
                    pslice = ps[:, v * OUT_SZ : (v + 1) * OUT_SZ]
                    nc.tensor.matmul(
                        pslice,
                        lhsT=xt[:, t * 128 : (t + 1) * 128],
                        rhs=wt[:, 0:OUT_SZ],
                        start=True,
                        stop=False,
                    )
                    nc.tensor.matmul(
                        pslice,
                        lhsT=xt[:, N + t * 128 : N + (t + 1) * 128],
                        rhs=wt[:, OUT_SZ : 2 * OUT_SZ],
                        start=False,
                        stop=True,
                    )
                # evacuate bank + bias add, bf16 out
                o0 = u * PSW * OUT_SZ
                nc.vector.tensor_add(
                    og[:, o0 : o0 + PSW * OUT_SZ].rearrange(
                        "p (t o) -> p t o", o=OUT_SZ
                    ),
                    ps[:].rearrange("p (t o) -> p t o", o=OUT_SZ),
                    bias3,
                )
            # store GROUP n-tiles in one DMA on the ACT HWDGE ring.
            # out DRAM layout is partition-major [b, p, t*OUT+o] so each
            # partition writes one contiguous GROUP*OUT*2-byte run; host
            # un-permutes to [b, n, o].
            nc.scalar.dma_start(
                out=out[b, :, g * GROUP * OUT_SZ : (g + 1) * GROUP * OUT_SZ],
                in_=og[:],
            )


def build_nc(niter=1):
    """Build + compile the per-core Bass graph (same graph on all 8 cores).

    niter > 1 wraps the workload in an on-device For_i loop — used only for
    timing (amortizes host/tunnel dispatch overhead over many repeats).
    """
    key = ("nc", niter)
    if key in _CACHE:
        return _CACHE[key]

    import contextlib

    import concourse.mybir as mybir
    import concourse.tile as tile
    from concourse import bacc

    nc = bacc.Bacc("TRN2", target_bir_lowering=False, debug=False)
    bf16 = mybir.dt.bfloat16
    f32 = mybir.dt.float32

    xT = nc.declare_dram_parameter("xT", [BPC, IN_SZ, N], bf16, isOutput=False)
    w = nc.declare_dram_parameter("w", [BPC, IN_SZ, OUT_SZ], bf16, isOutput=False)
    biasb = nc.declare_dram_parameter("biasb", [128, BPC * OUT_SZ], f32, isOutput=False)
    # partition-major output: out[b, p, t*OUT+o] = result[b, t*128+p, o]
    out = nc.declare_dram_parameter("out", [BPC, 128, NT * OUT_SZ], bf16, isOutput=True)

    with tile.TileContext(nc) as tc:
        with (
            tc.tile_pool(name="bias", bufs=1) as biasp,
            tc.tile_pool(name="xp", bufs=3) as xp,
            tc.tile_pool(name="wp", bufs=3) as wp,
            tc.tile_pool(name="op", bufs=4) as op,
            tc.tile_pool(name="psum", bufs=6, space="PSUM") as psum,
        ):
            bias_sb = biasp.tile([128, BPC * OUT_SZ], f32, tag="bias")
            nc.sync.dma_start(out=bias_sb[:], in_=biasb[:])

            ctx = tc.For_i(0, niter, 1) if niter > 1 else contextlib.nullcontext()
            with ctx:
                _emit_body(nc, xT, w, out, bias_sb, xp, wp, op, psum)

    nc.compile()
    _CACHE[key] = nc
    return nc


def prep_in_maps(x, indices, weights_U, weights_V, bias):
    """Host-side marshalling: gather/synthesize per-batch weights, transpose
    x per batch, cast to bf16, shard along B."""
    bf16 = _bf16()
    x = np.asarray(x)
    idx = np.asarray(indices).astype(np.int64)
    U = np.asarray(weights_U, dtype=np.float32)
    V = np.asarray(weights_V, dtype=np.float32)
    bias = np.asarray(bias, dtype=np.float32)

    W = (U[idx] @ V).reshape(B, IN_SZ, OUT_SZ).astype(bf16)  # [B, in, out]
    xT = np.ascontiguousarray(x.transpose(0, 2, 1)).astype(bf16)  # [B, in, n]
    bias_sel = bias[idx][:, 0, :]  # [B, out] f32

    in_maps = []
    for c in range(N_CORES):
        s = slice(c * BPC, (c + 1) * BPC)
        bias_flat = bias_sel[s].reshape(1, BPC * OUT_SZ)  # [1, 32*256]
        bias_bc = np.ascontiguousarray(
            np.broadcast_to(bias_flat, (128, BPC * OUT_SZ)), dtype=np.float32
        )
        in_maps.append({"xT": xT[s], "w": W[s], "biasb": bias_bc})
    return in_maps


def assemble_output(results):
    out = np.concatenate(
        [np.asarray(results[c]["out"], dtype=np.float32) for c in range(N_CORES)],
        axis=0,
    )
    # [B, p, t*OUT+o] -> [B, n=t*128+p, o]
    out = out.reshape(B, 128, NT, OUT_SZ).transpose(0, 2, 1, 3).reshape(B, N, OUT_SZ)
    return np.ascontiguousarray(out)  # [B, N, OUT] f32


def kernel(x, indices, weights_U, weights_V, bias):
    from concourse import bass2jax

    nc = build_nc()
    in_maps = prep_in_maps(x, indices, weights_U, weights_V, bias)
    results = bass2jax.run_bass_via_pjrt(nc, in_maps, n_cores=N_CORES)
    return assemble_output(results)


# revision 31
# speedup vs baseline: 1.0835x; 1.0835x over previous
"""Trainium2 Bass kernel for AdaptiveLRLinearWithChannel (moe_routing).

Reference math:
    w    = (weights_U[indices] @ weights_V).reshape(B, IN, OUT)
    out  = einsum('bni,bio->bno', x, w) + bias[indices]

Strategy (8 NeuronCores, data-parallel over B):
  - Shard B=256 into 8 x 32 batches; U/V/bias are folded on host into
    per-batch weight matrices W[b] and bias rows (host marshalling only; all
    O(B*N*IN*OUT) FLOPs run on device). x is re-laid out as xT[b] = x[b].T so
    the contraction dim (IN) lands on SBUF partitions; x/W cast to bf16
    (matmul accumulates in f32 PSUM; rel err ~3e-3).
  - The kernel computes outT[b] = W[b].T @ x[b].T tile-by-tile: the stationary
    operand is a W chunk (reused across 4 moving passes of 512 columns of
    xT), so the output lands transposed ([o, n]) in PSUM. That puts the bias
    on the partition axis, so PSUM evacuation (+bias, ->bf16) is split
    between VectorE (tensor_scalar_add) and ScalarE (activation Copy+bias),
    and the bias input is 32KB instead of MBs.
  - Outputs are stored partition-major ([b, o-chunk, o_lane, n]); the host
    un-permutes to [b, n, o]. Loads run on the SP HWDGE ring, stores on the
    ACT ring.
"""

import sys

for _p in ("/opt/trn_rl_repo",):
    if _p not in sys.path:
        sys.path.insert(0, _p)

import numpy as np

B = 256
N = 2048
IN_SZ = 256
OUT_SZ = 256
N_CORES = 8
BPC = B // N_CORES  # 32 batches per core
NSL = 4  # moving n-slices per (batch, o-chunk); each 512 wide
NSW = N // NSL  # 512
XBUFS = 5
OBUFS = 6
PBUFS = 8
STAGGERED = False  # staggered_reset on the timing For_i loop

_CACHE = {}


def _bf16():
    import ml_dtypes

    return ml_dtypes.bfloat16


def _emit_body(nc, xT, w, out, bias_sb, xp, wp, op, psum):
    import concourse.mybir as mybir

    bf16 = mybir.dt.bfloat16
    f32 = mybir.dt.float32
    Copy = mybir.ActivationFunctionType.Identity

    # wt_all[p, b*2*OUT + ci*OUT + o] = w[b, 128*ci + p, o] — one 4MB DMA
    wt_all = wp.tile([128, BPC * 2 * OUT_SZ], bf16, tag="wt_all")
    nc.sync.dma_start(
        out=wt_all[:].rearrange("p (b c o) -> p b c o", b=BPC, c=2),
        in_=w[:].rearrange("b (c p) o -> p b c o", p=128),
    )

    for b in range(BPC):
        # xt[p, ci*N + n] = xT[b, 128*ci + p, n]   (ci = K-chunk)
        xt = xp.tile([128, 2 * N], bf16, tag="xt")
        nc.sync.dma_start(
            out=xt[:].rearrange("p (c n) -> p c n", c=2),
            in_=xT[b].rearrange("(c p) n -> p c n", p=128),
        )
        for co in range(2):
            og = op.tile([128, N], bf16, tag="og")
            pss = [
                psum.tile([128, NSW], f32, tag="ps", name=f"ps{s}")
                for s in range(NSL)
            ]
            for ci in range(2):
                base = b * 2 * OUT_SZ + ci * OUT_SZ + co * 128
                lhsT = wt_all[:, base : base + 128]
                for s in range(NSL):
                    nc.tensor.matmul(
                        pss[s][:],
                        lhsT=lhsT,
                        rhs=xt[:, ci * N + s * NSW : ci * N + (s + 1) * NSW],
                        start=(ci == 0),
                        stop=(ci == 1),
                    )
            bias_col = bias_sb[:, b * 2 + co : b * 2 + co + 1]  # [128, 1] f32
            for s in range(NSL):
                dst = og[:, s * NSW : (s + 1) * NSW]
                if s % 2 == 0:
                    nc.vector.tensor_scalar_add(dst, pss[s][:], bias_col)
                else:
                    nc.scalar.activation(dst, pss[s][:], Copy, bias=bias_col)
            # store [o_lane=128, n=2048] bf16 on the ACT HWDGE ring;
            # 4KB contiguous per partition. Host un-permutes.
            nc.scalar.dma_start(out=out[b, co], in_=og[:])


def build_nc(niter=1):
    """Build + compile the per-core Bass graph (same graph on all 8 cores).

    niter > 1 wraps the workload in an on-device For_i loop — used only for
    timing (amortizes host/tunnel dispatch overhead over many repeats).
    """
    key = ("nc", niter)
    if key in _CACHE:
        return _CACHE[key]

    import contextlib

    import concourse.mybir as mybir
    import concourse.tile as tile
    from concourse import bacc

    nc = bacc.Bacc("TRN2", target_bir_lowering=False, debug=False)
    bf16 = mybir.dt.bfloat16
    f32 = mybir.dt.float32

    xT = nc.declare_dram_parameter("xT", [BPC, IN_SZ, N], bf16, isOutput=False)
    w = nc.declare_dram_parameter("w", [BPC, IN_SZ, OUT_SZ], bf16, isOutput=False)
    # biasb[p, b*2+co] = bias_sel[b, co*128 + p]
    biasb = nc.declare_dram_parameter("biasb", [128, BPC * 2], f32, isOutput=False)
    # partition-major transposed output: out[b, co, p, n] = result[b, n, co*128+p]
    out = nc.declare_dram_parameter("out", [BPC, 2, 128, N], bf16, isOutput=True)
    nit = (
        nc.declare_dram_parameter("nit", [1, 1], mybir.dt.int32, isOutput=False)
        if niter == "dyn"
        else None
    )

    with tile.TileContext(nc) as tc:
        with (
            tc.tile_pool(name="bias", bufs=1) as biasp,
            tc.tile_pool(name="xp", bufs=XBUFS) as xp,
            tc.tile_pool(name="wp", bufs=2) as wp,
            tc.tile_pool(name="op", bufs=OBUFS) as op,
            tc.tile_pool(name="psum", bufs=PBUFS, space="PSUM") as psum,
        ):
            bias_sb = biasp.tile([128, BPC * 2], f32, tag="bias")
            nc.sync.dma_start(out=bias_sb[:], in_=biasb[:])

            if niter == "dyn":
                nit_tile = biasp.tile([1, 1], mybir.dt.int32, tag="nit")
                nc.sync.dma_start(out=nit_tile[:], in_=nit[:])
                nval = nc.values_load(nit_tile[0:1, 0:1], min_val=1, max_val=1 << 20)
                ctx = tc.For_i(0, nval, 1, staggered_reset=STAGGERED)
            elif niter > 1:
                ctx = tc.For_i(0, niter, 1, staggered_reset=STAGGERED)
            else:
                ctx = contextlib.nullcontext()
            with ctx:
                _emit_body(nc, xT, w, out, bias_sb, xp, wp, op, psum)

    nc.compile()
    _CACHE[key] = nc
    return nc


def prep_in_maps(x, indices, weights_U, weights_V, bias):
    """Host-side marshalling: gather/synthesize per-batch weights, transpose
    x per batch, cast to bf16, shard along B."""
    bf16 = _bf16()
    x = np.asarray(x)
    idx = np.asarray(indices).astype(np.int64)
    U = np.asarray(weights_U, dtype=np.float32)
    V = np.asarray(weights_V, dtype=np.float32)
    bias = np.asarray(bias, dtype=np.float32)

    W = (U[idx] @ V).reshape(B, IN_SZ, OUT_SZ).astype(bf16)  # [B, in, out]
    xT = np.ascontiguousarray(x.transpose(0, 2, 1)).astype(bf16)  # [B, in, n]
    bias_sel = bias[idx][:, 0, :]  # [B, out] f32

    in_maps = []
    for c in range(N_CORES):
        s = slice(c * BPC, (c + 1) * BPC)
        # [128, BPC*2]: biasb[p, b*2+co] = bias_sel[b, co*128+p]
        bias_pm = np.ascontiguousarray(
            bias_sel[s].reshape(BPC, 2, 128).transpose(2, 0, 1).reshape(128, BPC * 2),
            dtype=np.float32,
        )
        in_maps.append({"xT": xT[s], "w": W[s], "biasb": bias_pm})
    return in_maps


def assemble_output(results):
    out = np.concatenate(
        [np.asarray(results[c]["out"], dtype=np.float32) for c in range(N_CORES)],
        axis=0,
    )
    # [B, co, p, n] -> [B, n, o=co*128+p]
    out = out.transpose(0, 3, 1, 2).reshape(B, N, OUT_SZ)
    return np.ascontiguousarray(out)


def kernel(x, indices, weights_U, weights_V, bias):
    from concourse import bass2jax

    nc = build_nc()
    in_maps = prep_in_maps(x, indices, weights_U, weights_V, bias)
    results = bass2jax.run_bass_via_pjrt(nc, in_maps, n_cores=N_CORES)
    return assemble_output(results)


# revision 37
# speedup vs baseline: 1.8103x; 1.6708x over previous
"""Trainium2 Bass kernel for AdaptiveLRLinearWithChannel (moe_routing).

Reference math:
    w    = (weights_U[indices] @ weights_V).reshape(B, IN, OUT)
    out  = einsum('bni,bio->bno', x, w) + bias[indices]

Strategy (8 NeuronCores, data-parallel over B):
  - Shard B=256 into 8 x 32 batches; U/V/bias are folded on host into
    per-batch weight matrices W[b] and bias rows (host marshalling only; all
    O(B*N*IN*OUT) FLOPs run on device). x is re-laid out as xT[b] = x[b].T so
    the contraction dim (IN) lands on SBUF partitions; x/W cast to bf16
    (matmul accumulates in f32 PSUM; rel err ~3e-3).
  - The kernel computes outT[b] = W[b].T @ x[b].T tile-by-tile: the stationary
    operand is a W chunk (reused across 4 moving passes of 512 columns of
    xT), so the output lands transposed ([o, n]) in PSUM. That puts the bias
    on the partition axis, so PSUM evacuation (+bias, ->bf16) is split
    between VectorE (tensor_scalar_add) and ScalarE (activation Copy+bias),
    and the bias input is 32KB instead of MBs.
  - Outputs are stored partition-major ([b, o-chunk, o_lane, n]); the host
    un-permutes to [b, n, o]. Loads run on the SP HWDGE ring, stores on the
    ACT ring.
"""

import sys

for _p in ("/opt/trn_rl_repo",):
    if _p not in sys.path:
        sys.path.insert(0, _p)

import numpy as np

B = 256
N = 2048
IN_SZ = 256
OUT_SZ = 256
N_CORES = 8
BPC = B // N_CORES  # 32 batches per core
NSL = 4  # moving n-slices per (batch, o-chunk); each 512 wide
NSW = N // NSL  # 512
XBUFS = 8
OBUFS = 8
PBUFS = 8
STAGGERED = True  # staggered_reset on the timing For_i loop

_CACHE = {}


def _bf16():
    import ml_dtypes

    return ml_dtypes.bfloat16


def _emit_body(nc, xT, w, out, bias_sb, xp, wp, op, psum):
    import concourse.mybir as mybir

    bf16 = mybir.dt.bfloat16
    f32 = mybir.dt.float32
    Copy = mybir.ActivationFunctionType.Identity

    # W loads chunked (4 batches per chunk), interleaved with the batch loop
    # so batch 0's matmuls only wait on chunk 0, not the whole 4MB.
    WCH = 4  # batches per W chunk
    wts = {}

    for b in range(BPC):
        if b % WCH == 0:
            j = b // WCH
            wt_j = wp.tile(
                [128, WCH * 2 * OUT_SZ], bf16, tag=f"wt{j}", name=f"wt{j}"
            )
            # wt_j[p, bb*2*OUT + ci*OUT + o] = w[j*WCH + bb, 128*ci + p, o]
            nc.sync.dma_start(
                out=wt_j[:].rearrange("p (bb c o) -> p bb c o", bb=WCH, c=2),
                in_=w[j * WCH : (j + 1) * WCH].rearrange(
                    "bb (c p) o -> p bb c o", p=128
                ),
            )
            wts[j] = wt_j
        # xt[p, ci*N + n] = xT[b, 128*ci + p, n]   (ci = K-chunk)
        xt = xp.tile([128, 2 * N], bf16, tag="xt")
        nc.sync.dma_start(
            out=xt[:].rearrange("p (c n) -> p c n", c=2),
            in_=xT[b].rearrange("(c p) n -> p c n", p=128),
        )
        for co in range(2):
            og = op.tile([128, N], bf16, tag="og")
            pss = [
                psum.tile([128, NSW], f32, tag="ps", name=f"ps{s}")
                for s in range(NSL)
            ]
            for ci in range(2):
                base = (b % WCH) * 2 * OUT_SZ + ci * OUT_SZ + co * 128
                lhsT = wts[b // WCH][:, base : base + 128]
                for s in range(NSL):
                    nc.tensor.matmul(
                        pss[s][:],
                        lhsT=lhsT,
                        rhs=xt[:, ci * N + s * NSW : ci * N + (s + 1) * NSW],
                        start=(ci == 0),
                        stop=(ci == 1),
                    )
            bias_col = bias_sb[:, b * 2 + co : b * 2 + co + 1]  # [128, 1] f32
            for s in range(NSL):
                dst = og[:, s * NSW : (s + 1) * NSW]
                if s % 2 == 0:
                    nc.vector.tensor_scalar_add(dst, pss[s][:], bias_col)
                else:
                    nc.scalar.activation(dst, pss[s][:], Copy, bias=bias_col)
            # store [o_lane=128, n=2048] bf16 on the ACT HWDGE ring;
            # 4KB contiguous per partition. Host un-permutes.
            nc.scalar.dma_start(out=out[b, co], in_=og[:])


def build_nc(niter=1):
    """Build + compile the per-core Bass graph (same graph on all 8 cores).

    niter > 1 wraps the workload in an on-device For_i loop — used only for
    timing (amortizes host/tunnel dispatch overhead over many repeats).
    """
    key = ("nc", niter)
    if key in _CACHE:
        return _CACHE[key]

    import contextlib

    import concourse.mybir as mybir
    import concourse.tile as tile
    from concourse import bacc

    nc = bacc.Bacc("TRN2", target_bir_lowering=False, debug=False)
    bf16 = mybir.dt.bfloat16
    f32 = mybir.dt.float32

    xT = nc.declare_dram_parameter("xT", [BPC, IN_SZ, N], bf16, isOutput=False)
    w = nc.declare_dram_parameter("w", [BPC, IN_SZ, OUT_SZ], bf16, isOutput=False)
    # biasb[p, b*2+co] = bias_sel[b, co*128 + p]
    biasb = nc.declare_dram_parameter("biasb", [128, BPC * 2], f32, isOutput=False)
    # partition-major transposed output: out[b, co, p, n] = result[b, n, co*128+p]
    out = nc.declare_dram_parameter("out", [BPC, 2, 128, N], bf16, isOutput=True)
    nit = (
        nc.declare_dram_parameter("nit", [1, 1], mybir.dt.int32, isOutput=False)
        if niter == "dyn"
        else None
    )

    with tile.TileContext(nc) as tc:
        with (
            tc.tile_pool(name="bias", bufs=1) as biasp,
            tc.tile_pool(name="xp", bufs=XBUFS) as xp,
            tc.tile_pool(name="wp", bufs=1) as wp,
            tc.tile_pool(name="op", bufs=OBUFS) as op,
            tc.tile_pool(name="psum", bufs=PBUFS, space="PSUM") as psum,
        ):
            bias_sb = biasp.tile([128, BPC * 2], f32, tag="bias")
            nc.sync.dma_start(out=bias_sb[:], in_=biasb[:])

            if niter == "dyn":
                nit_tile = biasp.tile([1, 1], mybir.dt.int32, tag="nit")
                nc.sync.dma_start(out=nit_tile[:], in_=nit[:])
                nval = nc.values_load(
                    nit_tile[0:1, 0:1],
                    min_val=1,
                    max_val=1 << 20,
                    skip_runtime_bounds_check=True,
                )
                ctx = tc.For_i(0, nval, 1, staggered_reset=STAGGERED)
            elif niter > 1:
                ctx = tc.For_i(0, niter, 1, staggered_reset=STAGGERED)
            else:
                ctx = contextlib.nullcontext()
            with ctx:
                _emit_body(nc, xT, w, out, bias_sb, xp, wp, op, psum)

    nc.compile()
    _CACHE[key] = nc
    return nc


def prep_in_maps(x, indices, weights_U, weights_V, bias):
    """Host-side marshalling: gather/synthesize per-batch weights, transpose
    x per batch, cast to bf16, shard along B."""
    bf16 = _bf16()
    x = np.asarray(x)
    idx = np.asarray(indices).astype(np.int64)
    U = np.asarray(weights_U, dtype=np.float32)
    V = np.asarray(weights_V, dtype=np.float32)
    bias = np.asarray(bias, dtype=np.float32)

    W = (U[idx] @ V).reshape(B, IN_SZ, OUT_SZ).astype(bf16)  # [B, in, out]
    xT = np.ascontiguousarray(x.transpose(0, 2, 1)).astype(bf16)  # [B, in, n]
    bias_sel = bias[idx][:, 0, :]  # [B, out] f32

    in_maps = []
    for c in range(N_CORES):
        s = slice(c * BPC, (c + 1) * BPC)
        # [128, BPC*2]: biasb[p, b*2+co] = bias_sel[b, co*128+p]
        bias_pm = np.ascontiguousarray(
            bias_sel[s].reshape(BPC, 2, 128).transpose(2, 0, 1).reshape(128, BPC * 2),
            dtype=np.float32,
        )
        in_maps.append({"xT": xT[s], "w": W[s], "biasb": bias_pm})
    return in_maps


def assemble_output(results):
    out = np.concatenate(
        [np.asarray(results[c]["out"], dtype=np.float32) for c in range(N_CORES)],
        axis=0,
    )
    # [B, co, p, n] -> [B, n, o=co*128+p]
    out = out.transpose(0, 3, 1, 2).reshape(B, N, OUT_SZ)
    return np.ascontiguousarray(out)


def kernel(x, indices, weights_U, weights_V, bias):
    from concourse import bass2jax

    nc = build_nc()
    in_maps = prep_in_maps(x, indices, weights_U, weights_V, bias)
    results = bass2jax.run_bass_via_pjrt(nc, in_maps, n_cores=N_CORES)
    return assemble_output(results)


# revision 38
# speedup vs baseline: 1.8863x; 1.0419x over previous
"""Trainium2 Bass kernel for AdaptiveLRLinearWithChannel (moe_routing).

Reference math:
    w    = (weights_U[indices] @ weights_V).reshape(B, IN, OUT)
    out  = einsum('bni,bio->bno', x, w) + bias[indices]

Strategy (8 NeuronCores, data-parallel over B):
  - Shard B=256 into 8 x 32 batches. Host marshalling only: gather U[idx] /
    bias[idx], re-lay x out as xT[b] = x[b].T so the contraction dim (IN)
    lands on SBUF partitions, cast to bf16 (matmul accumulates in f32 PSUM;
    rel err ~3e-3). All O(B*N*IN*OUT) FLOPs and the low-rank weight
    synthesis w[b] = sum_r U[b,r] * V_r run on device: VectorE fma's the 4
    rank-1 components into each per-batch weight tile (saves 4MB/core of
    DMA vs shipping W; the kernel is HBM-bandwidth-bound).
  - The kernel computes outT[b] = W[b].T @ x[b].T tile-by-tile: the stationary
    operand is a W chunk (reused across 4 moving passes of 512 columns of
    xT), so the output lands transposed ([o, n]) in PSUM. That puts the bias
    on the partition axis, so PSUM evacuation (+bias, ->bf16) is split
    between VectorE (tensor_scalar_add) and ScalarE (activation Copy+bias),
    and the bias input is 32KB instead of MBs.
  - Outputs are stored partition-major ([b, o-chunk, o_lane, n]); the host
    un-permutes to [b, n, o]. Loads run on the SP HWDGE ring, stores on the
    ACT ring.
"""

import sys

for _p in ("/opt/trn_rl_repo",):
    if _p not in sys.path:
        sys.path.insert(0, _p)

import numpy as np

B = 256
N = 2048
IN_SZ = 256
OUT_SZ = 256
N_CORES = 8
BPC = B // N_CORES  # 32 batches per core
NSL = 4  # moving n-slices per (batch, o-chunk); each 512 wide
NSW = N // NSL  # 512
XBUFS = 8
OBUFS = 8
PBUFS = 8
STAGGERED = True  # staggered_reset on the timing For_i loop

_CACHE = {}


def _bf16():
    import ml_dtypes

    return ml_dtypes.bfloat16


def _emit_body(nc, xT, vdev_sb, ubc_sb, out, bias_sb, xp, wp, op, psum):
    import concourse.mybir as mybir

    bf16 = mybir.dt.bfloat16
    f32 = mybir.dt.float32
    Copy = mybir.ActivationFunctionType.Identity

    # On-device low-rank W synthesis: w[b, 128*ci+p, o] = sum_r U[b,r] *
    # V[r, (128*ci+p)*256+o].  vdev_sb[p, (ci*4+r)*256+o] holds V chunks;
    # ubc_sb[p, b*4+r] holds U[b, r] replicated across partitions.  DVE does
    # 4 mul/fma passes per [128, 256] chunk into a bf16 wt tile.
    mult = mybir.AluOpType.mult
    add = mybir.AluOpType.add

    for b in range(BPC):
        wt_b = wp.tile([128, 2 * OUT_SZ], bf16, tag="wt", name=f"wt{b}")
        acc = wp.tile([128, OUT_SZ], f32, tag="acc", name=f"acc{b}")
        for ci in range(2):
            dst = wt_b[:, ci * OUT_SZ : (ci + 1) * OUT_SZ]
            for r in range(4):
                vsl = vdev_sb[:, (ci * 4 + r) * OUT_SZ : (ci * 4 + r + 1) * OUT_SZ]
                usl = ubc_sb[:, b * 4 + r : b * 4 + r + 1]
                if r == 0:
                    nc.vector.tensor_scalar_mul(acc[:], vsl, usl)
                elif r < 3:
                    nc.vector.scalar_tensor_tensor(acc[:], vsl, usl, acc[:], mult, add)
                else:
                    nc.vector.scalar_tensor_tensor(dst, vsl, usl, acc[:], mult, add)
        # xt[p, ci*N + n] = xT[b, 128*ci + p, n]   (ci = K-chunk)
        xt = xp.tile([128, 2 * N], bf16, tag="xt")
        nc.sync.dma_start(
            out=xt[:].rearrange("p (c n) -> p c n", c=2),
            in_=xT[b].rearrange("(c p) n -> p c n", p=128),
        )
        for co in range(2):
            og = op.tile([128, N], bf16, tag="og")
            pss = [
                psum.tile([128, NSW], f32, tag="ps", name=f"ps{s}")
                for s in range(NSL)
            ]
            for ci in range(2):
                base = ci * OUT_SZ + co * 128
                lhsT = wt_b[:, base : base + 128]
                for s in range(NSL):
                    nc.tensor.matmul(
                        pss[s][:],
                        lhsT=lhsT,
                        rhs=xt[:, ci * N + s * NSW : ci * N + (s + 1) * NSW],
                        start=(ci == 0),
                        stop=(ci == 1),
                    )
            bias_col = bias_sb[:, b * 2 + co : b * 2 + co + 1]  # [128, 1] f32
            for s in range(NSL):
                dst = og[:, s * NSW : (s + 1) * NSW]
                if s % 2 == 0:
                    nc.vector.tensor_scalar_add(dst, pss[s][:], bias_col)
                else:
                    nc.scalar.activation(dst, pss[s][:], Copy, bias=bias_col)
            # store [o_lane=128, n=2048] bf16 on the ACT HWDGE ring;
            # 4KB contiguous per partition. Host un-permutes.
            nc.scalar.dma_start(out=out[b, co], in_=og[:])


def build_nc(niter=1):
    """Build + compile the per-core Bass graph (same graph on all 8 cores).

    niter > 1 wraps the workload in an on-device For_i loop — used only for
    timing (amortizes host/tunnel dispatch overhead over many repeats).
    """
    key = ("nc", niter)
    if key in _CACHE:
        return _CACHE[key]

    import contextlib

    import concourse.mybir as mybir
    import concourse.tile as tile
    from concourse import bacc

    nc = bacc.Bacc("TRN2", target_bir_lowering=False, debug=False)
    bf16 = mybir.dt.bfloat16
    f32 = mybir.dt.float32

    xT = nc.declare_dram_parameter("xT", [BPC, IN_SZ, N], bf16, isOutput=False)
    vdev = nc.declare_dram_parameter("vdev", [128, 2 * 4 * OUT_SZ], bf16, isOutput=False)
    ubc = nc.declare_dram_parameter("ubc", [128, BPC * 4], f32, isOutput=False)
    # biasb[p, b*2+co] = bias_sel[b, co*128 + p]
    biasb = nc.declare_dram_parameter("biasb", [128, BPC * 2], f32, isOutput=False)
    # partition-major transposed output: out[b, co, p, n] = result[b, n, co*128+p]
    out = nc.declare_dram_parameter("out", [BPC, 2, 128, N], bf16, isOutput=True)
    nit = (
        nc.declare_dram_parameter("nit", [1, 1], mybir.dt.int32, isOutput=False)
        if niter == "dyn"
        else None
    )

    with tile.TileContext(nc) as tc:
        with (
            tc.tile_pool(name="bias", bufs=1) as biasp,
            tc.tile_pool(name="xp", bufs=XBUFS) as xp,
            tc.tile_pool(name="wp", bufs=3) as wp,
            tc.tile_pool(name="op", bufs=OBUFS) as op,
            tc.tile_pool(name="psum", bufs=PBUFS, space="PSUM") as psum,
        ):
            bias_sb = biasp.tile([128, BPC * 2], f32, tag="bias")
            nc.sync.dma_start(out=bias_sb[:], in_=biasb[:])
            vdev_sb = biasp.tile([128, 2 * 4 * OUT_SZ], bf16, tag="vdev")
            nc.sync.dma_start(out=vdev_sb[:], in_=vdev[:])
            ubc_sb = biasp.tile([128, BPC * 4], f32, tag="ubc")
            nc.sync.dma_start(out=ubc_sb[:], in_=ubc[:])

            if niter == "dyn":
                nit_tile = biasp.tile([1, 1], mybir.dt.int32, tag="nit")
                nc.sync.dma_start(out=nit_tile[:], in_=nit[:])
                nval = nc.values_load(
                    nit_tile[0:1, 0:1],
                    min_val=1,
                    max_val=1 << 20,
                    skip_runtime_bounds_check=True,
                )
                ctx = tc.For_i(0, nval, 1, staggered_reset=STAGGERED)
            elif niter > 1:
                ctx = tc.For_i(0, niter, 1, staggered_reset=STAGGERED)
            else:
                ctx = contextlib.nullcontext()
            with ctx:
                _emit_body(nc, xT, vdev_sb, ubc_sb, out, bias_sb, xp, wp, op, psum)

    nc.compile()
    _CACHE[key] = nc
    return nc


def prep_in_maps(x, indices, weights_U, weights_V, bias):
    """Host-side marshalling: gather/synthesize per-batch weights, transpose
    x per batch, cast to bf16, shard along B."""
    bf16 = _bf16()
    x = np.asarray(x)
    idx = np.asarray(indices).astype(np.int64)
    U = np.asarray(weights_U, dtype=np.float32)
    V = np.asarray(weights_V, dtype=np.float32)
    bias = np.asarray(bias, dtype=np.float32)

    xT = np.ascontiguousarray(x.transpose(0, 2, 1)).astype(bf16)  # [B, in, n]
    bias_sel = bias[idx][:, 0, :]  # [B, out] f32
    U_sel = U[idx]  # [B, 4] f32
    # vdev[p, (ci*4+r)*256+o] = V[r, (128*ci+p)*256+o]
    V4 = V.reshape(4, 2, 128, OUT_SZ)  # [r, ci, p, o]
    vdev = np.ascontiguousarray(
        V4.transpose(2, 1, 0, 3).reshape(128, 2 * 4 * OUT_SZ)
    ).astype(bf16)

    in_maps = []
    for c in range(N_CORES):
        s = slice(c * BPC, (c + 1) * BPC)
        # [128, BPC*2]: biasb[p, b*2+co] = bias_sel[b, co*128+p]
        bias_pm = np.ascontiguousarray(
            bias_sel[s].reshape(BPC, 2, 128).transpose(2, 0, 1).reshape(128, BPC * 2),
            dtype=np.float32,
        )
        ubc = np.ascontiguousarray(
            np.broadcast_to(U_sel[s].reshape(1, BPC * 4), (128, BPC * 4)),
            dtype=np.float32,
        )
        in_maps.append({"xT": xT[s], "vdev": vdev, "ubc": ubc, "biasb": bias_pm})
    return in_maps


def assemble_output(results):
    out = np.concatenate(
        [np.asarray(results[c]["out"], dtype=np.float32) for c in range(N_CORES)],
        axis=0,
    )
    # [B, co, p, n] -> [B, n, o=co*128+p]
    out = out.transpose(0, 3, 1, 2).reshape(B, N, OUT_SZ)
    return np.ascontiguousarray(out)


def kernel(x, indices, weights_U, weights_V, bias):
    from concourse import bass2jax

    nc = build_nc()
    in_maps = prep_in_maps(x, indices, weights_U, weights_V, bias)
    results = bass2jax.run_bass_via_pjrt(nc, in_maps, n_cores=N_CORES)
    return assemble_output(results)
